# revision 1
# baseline (speedup 1.0000x reference)
"""Trainium2 Bass kernel: 14-qubit data-reuploading quantum circuit actor.

Circuit per layer l (NL=5):
  for w in 0..13:  RY(in_scale[l,w]*x[:,w]) ; RZ(in_scale[l,w+14]*x[:,w]) on wire w
  for w in 0..13:  RZ(weights[l,w]) on wire w          (merged into input RZ)
  for w in 0..13:  RY(weights[l,w+14]) on wire w
  CNOT ring (i -> i+1 mod 14)
Then <Z_w> for w in 0..5, * action_scale + action_bias.

This environment's cost model (measured): per-call round-trip through the
axon tunnel ~65-100ms (dominant), input transfer ~10ms/MB, per-instruction
~10us, per-ELEMENT ~0.  So the kernel (a) minimizes instruction count,
(b) minimizes per-call bytes, (c) uses a persistent cached jax.jit
executor (a fresh jit per call costs ~165ms of retracing):

  - state: fp16 SBUF planes, complex-interleaved [128 batch-partitions,
    32768 floats] (float f = 2*amp + (0=re,1=im)), double-buffered A<->B:
    every gate reads one, writes the other.  No copy-backs, no deferred
    cosines (exact rotations keep |amp|<=1, fp16-safe).
  - custom DVE op ROT2: out = s0*in0 + s1*in1 (two per-partition scalars).
    The TTSS encoding needs in1 rank-1, so each wire's RY writes its
    halves compacted into scratch T/U (one staging copy, itself a ROT2),
    and the RZ reads T/U rank-1 and writes back to the state canonically.
    RY+RZ = 7 instrs/wire; RY_weight with ring-CNOT folded into write APs
    = 6 (less for wires 0/1).
  - CNOT(13,0) folded into next layer's RY(0) read APs; for the last
    layer folded into the sqsum (measurement) read APs.
  - input = 210 half-angles/row (f32); cos/sin/-sin computed ON-CHIP via
    add_range_wrap + Horner polynomials (the Sin activation LUT is only
    ~4e-3 accurate here, and tensor_tensor_reduce wedges the device -
    avoid both).
  - measurement: SQSUM custom op (re^2+im^2) -> 64 block sums -> signed
    contraction to the 6 <Z_w> on chip (output 6 floats/row).
  ~930 instructions per 128-row tile, 2 tiles per core (batch 2048 over
  8 cores = 256 rows/core).
"""

import os
import numpy as np

NQ = 14
NL = 5
OBS = 14
NA = 6
B = 2048
NCORES = 8
BPC = B // NCORES          # 256 batch rows per core
PT = 128                   # partitions (batch rows) per tile
NTILES = BPC // PT         # 2
NS = 1 << NQ               # 16384 amplitudes
F = 2 * NS                 # 32768 floats per row (complex-interleaved)
NANG = 3 * NL * NQ         # 210 half-angles per row: k = type*70 + l*14 + w
NCOLS = 3 * NANG           # 630 coef columns on-chip: [cos | sin | -sin]
# col(l, w, t): t in 0..8, type = t//3 (ry/rz/wy), kind = t%3 (c/s/ns)
RY_C, RY_S, RY_NS = 0, 1, 2
RZ_C, RZ_S, RZ_NS = 3, 4, 5
WY_C, WY_S, WY_NS = 6, 7, 8

# ---------------------------------------------------------------- host tables


def col(l, w, t):
    return (t % 3) * NANG + (t // 3) * (NL * NQ) + l * NQ + w


def a_table(x, input_scaling, weights):
    """(n, NANG) f32 of half-angles, k = type*70 + l*14 + w."""
    x = np.asarray(x, np.float64)
    isc = np.asarray(input_scaling, np.float64)
    wt = np.asarray(weights, np.float64)
    n = x.shape[0]
    xb = x[:, None, :]  # (n, 1, NQ) broadcast over layers
    tab = np.empty((n, 3, NL, NQ), np.float64)
    tab[:, 0] = isc[None, :, :NQ] * xb / 2.0
    tab[:, 1] = (isc[None, :, OBS:] * xb + wt[None, :, :NQ]) / 2.0
    tab[:, 2] = np.broadcast_to(wt[None, :, NQ:] / 2.0, (n, NL, NQ))
    return tab.reshape(n, NANG).astype(np.float32)


def coef_table(a):
    """(n, NCOLS) f32 [cos | sin | -sin] of the half-angle table (sim only;
    on device this is computed by wrap + Sin activations)."""
    a = np.asarray(a, np.float64)
    return np.concatenate(
        [np.cos(a), np.sin(a), -np.sin(a)], axis=1
    ).astype(np.float32)


def postprocess(s64, action_scale, action_bias):
    """s64: (n, 64) block sums (blocks = top-6 amp bits). -> (n, NA) f32.
    (Numpy-sim path; on device the sign contraction runs on-chip.)"""
    blk = np.arange(64)
    out = np.zeros((s64.shape[0], NA), np.float32)
    for w in range(NA):
        sign = 1.0 - 2.0 * ((blk >> (5 - w)) & 1)
        out[:, w] = s64 @ sign.astype(np.float32)
    return out * np.asarray(action_scale, np.float32) + np.asarray(
        action_bias, np.float32
    )


# ------------------------------------------------------------- gate schedule
# region = (buf, offset, dims); dims = tuple of (step, count), innermost
# last, in float-index space (f = 2*amp + comp).  Buffers: "A"/"B" full
# state planes, "T"/"U" 16384-float scratch (compacted wire halves; "T"
# doubles as the probability plane P at measurement), "S" the s64 output.
# ops:
#   ("rot2", dst, s0, s1, c0, c1): dst = c0*s0 + c1*s1.  The custom-DVE
#       TTSS encoding requires s1 to have a SINGLE free dim (rank-1).
#   ("cp", dst, src):              dst = src (staging copy)
#   ("sqsum", dst, s0, s1):        dst = s0^2 + s1^2 (rank-1 s1)
#   ("red", dst, src):             s64 block reduce


def _norm(dims):
    """drop unit dims, merge contiguous, assert rank<=2."""
    d = [(s, c) for s, c in dims if c != 1]
    out = []
    for s, c in d:
        if out and out[-1][0] == s * c:
            out[-1] = (s, c * out[-1][1])
        else:
            out.append((s, c))
    if not out:
        out = [(1, 1)]
    assert len(out) <= 2, out
    return tuple(out)


def _reg(buf, off, *dims):
    return (buf, off, _norm(dims))


class Sched:
    def __init__(self):
        self.ops = []
        self.cur = "A"

    def swap(self):
        self.cur = "B" if self.cur == "A" else "A"

    def rot2(self, dst, s0, s1, c0, c1):
        assert len(s1[2]) == 1, ("rot2 s1 must be rank-1", s1)
        self.ops.append(("rot2", dst, s0, s1, c0, c1))

    def cp(self, dst, src):
        self.ops.append(("cp", dst, src))

    # wire-0 gates: halves are contiguous -> rank-1, no staging ------------
    def ry0(self, cc, cs, cns):
        a, b = self.cur, "B" if self.cur == "A" else "A"
        X = _reg(a, 0, (1, NS))
        Y = _reg(a, NS, (1, NS))
        self.rot2(_reg(b, 0, (1, NS)), X, Y, cc, cns)
        self.rot2(_reg(b, NS, (1, NS)), X, Y, cs, cc)
        self.swap()

    def ry0_fold(self, cc, cs, cns):
        """RY(0) reading through the previous layer's CNOT(13, 0): stage the
        logical Y half into U piece-major (lsb0 | lsb1), then 4 rot2."""
        a, b = self.cur, "B" if self.cur == "A" else "A"
        d = ((4, 4096), (1, 2))
        E00, E01 = _reg(a, 0, *d), _reg(a, 2, *d)
        E10, E11 = _reg(a, 16384, *d), _reg(a, 16386, *d)
        U0, U1 = _reg("U", 0, (1, 8192)), _reg("U", 8192, (1, 8192))
        self.cp(U0, E10)  # logical Y, lsb=0
        self.cp(U1, E01)  # logical Y, lsb=1 (msb-flipped physically)
        B00, B01 = _reg(b, 0, *d), _reg(b, 2, *d)
        B10, B11 = _reg(b, 16384, *d), _reg(b, 16386, *d)
        self.rot2(B00, E00, U0, cc, cns)
        self.rot2(B10, E00, U0, cs, cc)
        self.rot2(B01, E11, U1, cc, cns)  # lsb=1: X lives msb-flipped
        self.rot2(B11, E11, U1, cs, cc)
        self.swap()

    def rz0(self, cc, cs, cns):
        a, b = self.cur, "B" if self.cur == "A" else "A"

        def r(buf, off):
            return _reg(buf, off, (2, 8192))

        self.rot2(r(b, 0), r(a, 0), r(a, 1), cc, cs)
        self.rot2(r(b, 1), r(a, 0), r(a, 1), cns, cc)
        self.rot2(r(b, NS), r(a, NS), r(a, NS + 1), cc, cns)
        self.rot2(r(b, NS + 1), r(a, NS), r(a, NS + 1), cs, cc)
        self.swap()

    # generic wires: RY into scratch T/U, RZ from scratch back to state ----
    def ry_rz(self, w, ryc, rys, ryns, rzc, rzs, rzns):
        a, b = self.cur, "B" if self.cur == "A" else "A"
        sa = 1 << (13 - w)
        nb = 1 << w
        X = _reg(a, 0, (4 * sa, nb), (1, 2 * sa))
        Y = _reg(a, 2 * sa, (4 * sa, nb), (1, 2 * sa))
        T = _reg("T", 0, (1, NS))
        U = _reg("U", 0, (1, NS))
        self.cp(U, Y)
        self.rot2(T, X, U, ryc, ryns)   # X' compacted
        self.rot2(U, X, U, rys, ryc)    # Y' in place
        Tre, Tim = _reg("T", 0, (2, 8192)), _reg("T", 1, (2, 8192))
        Ure, Uim = _reg("U", 0, (2, 8192)), _reg("U", 1, (2, 8192))
        Xre = _reg(b, 0, (4 * sa, nb), (2, sa))
        Xim = _reg(b, 1, (4 * sa, nb), (2, sa))
        Yre = _reg(b, 2 * sa, (4 * sa, nb), (2, sa))
        Yim = _reg(b, 2 * sa + 1, (4 * sa, nb), (2, sa))
        self.rot2(Xre, Tre, Tim, rzc, rzs)
        self.rot2(Xim, Tre, Tim, rzns, rzc)
        self.rot2(Yre, Ure, Uim, rzc, rzns)
        self.rot2(Yim, Ure, Uim, rzs, rzc)
        self.swap()

    def ry_fold_cnot(self, t, cc, cs, cns):
        """RY_w(t) with CNOT(t-1, t) folded into write APs (t >= 1)."""
        a, b = self.cur, "B" if self.cur == "A" else "A"
        fst = 2 * (1 << (13 - t))
        pt = 2 * fst
        P2 = 2 * pt
        nb = 1 << (t - 1)

        def R(buf, ai, bi):
            return _reg(buf, ai * pt + bi * fst, (P2, nb), (1, fst))

        if t == 1:  # nb == 1: all regions rank-1, no staging
            s01, s11 = R(a, 0, 1), R(a, 1, 1)
        else:
            s01, s11 = _reg("T", 0, (1, 8192)), _reg("T", 8192, (1, 8192))
            self.cp(s01, R(a, 0, 1))
            self.cp(s11, R(a, 1, 1))
        self.rot2(R(b, 0, 0), R(a, 0, 0), s01, cc, cns)
        self.rot2(R(b, 0, 1), R(a, 0, 0), s01, cs, cc)
        self.rot2(R(b, 1, 1), R(a, 1, 0), s11, cc, cns)  # X' -> flipped
        self.rot2(R(b, 1, 0), R(a, 1, 0), s11, cs, cc)   # Y' -> flipped
        self.swap()

    def measurement(self):
        a = self.cur
        # P[amp] = |state[pi(amp)]|^2, pi = last layer's CNOT(13,0) perm.
        # P lives in T.
        self.ops.append(
            ("sqsum", _reg("T", 0, (2, 8192)),
             _reg(a, 0, (4, 8192)), _reg(a, 1, (4, 8192)))
        )
        self.ops.append(
            ("sqsum", _reg("T", 1, (2, 4096)),
             _reg(a, 16386, (4, 4096)), _reg(a, 16387, (4, 4096)))
        )
        self.ops.append(
            ("sqsum", _reg("T", 8193, (2, 4096)),
             _reg(a, 2, (4, 4096)), _reg(a, 3, (4, 4096)))
        )
        self.ops.append(
            ("red", ("S", 0, ((1, 64),)), ("T", 0, ((256, 64), (1, 256))))
        )


def build_schedule():
    S = Sched()
    for l in range(NL):
        for w in range(NQ):
            if w == 0:
                if l == 0:
                    S.ry0(col(l, 0, RY_C), col(l, 0, RY_S), col(l, 0, RY_NS))
                else:
                    S.ry0_fold(col(l, 0, RY_C), col(l, 0, RY_S), col(l, 0, RY_NS))
                S.rz0(col(l, 0, RZ_C), col(l, 0, RZ_S), col(l, 0, RZ_NS))
            else:
                S.ry_rz(
                    w,
                    col(l, w, RY_C), col(l, w, RY_S), col(l, w, RY_NS),
                    col(l, w, RZ_C), col(l, w, RZ_S), col(l, w, RZ_NS),
                )
        S.ry0(col(l, 0, WY_C), col(l, 0, WY_S), col(l, 0, WY_NS))
        for t in range(1, NQ):
            S.ry_fold_cnot(t, col(l, t, WY_C), col(l, t, WY_S), col(l, t, WY_NS))
    S.measurement()
    return S.ops


# ------------------------------------------------------------ numpy executor


def _indices(reg):
    _, off, dims = reg
    idx = np.array([0], np.int64)
    for st, ct in dims:
        idx = (idx[:, None] + (np.arange(ct, dtype=np.int64) * st)[None, :]).ravel()
    return off + idx


def simulate_numpy(a, fp16=True):
    """a: (n, NANG) f32 half-angle table -> (n, 64) block sums."""
    tab = coef_table(a)
    n = tab.shape[0]
    sdt = np.float16 if fp16 else np.float32
    bufs = {
        "A": np.zeros((n, F), sdt),
        "B": np.zeros((n, F), sdt),
        "T": np.zeros((n, NS), sdt),
        "U": np.zeros((n, NS), sdt),
        "S": np.zeros((n, 64), np.float32),
    }
    bufs["A"][:, 0] = 1.0
    A = tab
    for op in build_schedule():
        kind = op[0]
        if kind == "rot2":
            _, dst, s0, s1, c0, c1 = op
            v = (
                A[:, c0 : c0 + 1].astype(np.float32)
                * bufs[s0[0]][:, _indices(s0)].astype(np.float32)
                + A[:, c1 : c1 + 1].astype(np.float32)
                * bufs[s1[0]][:, _indices(s1)].astype(np.float32)
            )
            bufs[dst[0]][:, _indices(dst)] = v.astype(sdt)
        elif kind == "cp":
            _, dst, src = op
            bufs[dst[0]][:, _indices(dst)] = bufs[src[0]][:, _indices(src)]
        elif kind == "sqsum":
            _, dst, s0, s1 = op
            v = (
                bufs[s0[0]][:, _indices(s0)].astype(np.float32) ** 2
                + bufs[s1[0]][:, _indices(s1)].astype(np.float32) ** 2
            )
            bufs[dst[0]][:, _indices(dst)] = v.astype(sdt)
        elif kind == "red":
            _, dst, src = op
            v = bufs[src[0]][:, _indices(src)].astype(np.float32)
            bufs["S"][:, _indices(dst)] = v.reshape(n, 64, 256).sum(axis=2)
        else:
            raise ValueError(kind)
    return bufs["S"].copy()


# ------------------------------------------------------------------ bass side

_CUSTOM_OPS = {}


def _register_op(name, spec):
    from concourse.dve_uop import DveOpSpec
    from concourse.dve_spec import lower
    from concourse import dve_ops
    from concourse.dve_ops import DveOp, OPS

    for op in OPS:
        if op.name == name:
            return op
    row = dve_ops._CUSTOM_DVE_ROW_BASE + len(OPS)
    shas = {}
    for ver in ("v3", "v4"):
        shas[ver] = DveOpSpec(
            name=name, opcode=row, uops=lower(spec, ver=ver), rd1_en=True
        ).sha(ver)
    op = DveOp(name, spec, subdim=False, uops_sha=shas)
    OPS.append(op)
    dve_ops._SUB_OPCODE_FOR_NAME[name] = row
    dve_ops.CUSTOM_DVE_SPECS[name] = spec
    return op


def _get_custom_ops():
    """Register fused DVE ops (idempotent): ROT2 out = s0*in0 + s1*in1,
    SQSUM out = in0^2 + in1^2."""
    if _CUSTOM_OPS:
        return _CUSTOM_OPS
    from concourse.dve_spec import Spec, Src0, Src1, C0, C1, sq

    _CUSTOM_OPS["rot2"] = _register_op(
        "ROT2_ANT",
        Spec(
            body=Src0 * C0 + Src1 * C1,
            reference=lambda in0, in1, s0, s1, imm2: (
                np.asarray(in0, np.float32) * np.asarray(s0, np.float32)
                + np.asarray(in1, np.float32) * np.asarray(s1, np.float32)
            ).astype(np.float32),
        ),
    )
    _CUSTOM_OPS["sqsum"] = _register_op(
        "SQSUM_ANT",
        Spec(
            body=sq(Src0) + sq(Src1),
            reference=lambda in0, in1, s0, s1, imm2: (
                np.asarray(in0, np.float32) ** 2 + np.asarray(in1, np.float32) ** 2
            ).astype(np.float32),
        ),
    )
    return _CUSTOM_OPS


def _ap(bass_mod, tile_ap, reg):
    t = tile_ap.tensor
    part = list(tile_ap.ap)[0]
    dims = [[part[0], part[1]]] + [[s, c] for s, c in reg[2]]
    return bass_mod.AP(t, tile_ap.offset + reg[1], dims)


def build_bass():
    import concourse.bass as bass
    import concourse.mybir as mybir
    import concourse.tile as tile
    from concourse import bacc
    from contextlib import ExitStack

    f32 = mybir.dt.float32
    f16 = mybir.dt.float16
    PI = float(np.pi)
    nc = bacc.Bacc("TRN2", target_bir_lowering=False, debug=False)
    a_d = nc.dram_tensor("a", [BPC, NANG], f32, kind="ExternalInput").ap()
    out_d = nc.dram_tensor("out", [BPC, NA], f32, kind="ExternalOutput").ap()

    sched = build_schedule()
    cops = _get_custom_ops()

    with tile.TileContext(nc) as tc, ExitStack() as ctx:
        state_p = ctx.enter_context(tc.tile_pool(name="state", bufs=1))
        io_p = ctx.enter_context(tc.tile_pool(name="io", bufs=2))

        A_t = state_p.tile([PT, F], f16, tag="A")
        B_t = state_p.tile([PT, F], f16, tag="B")
        T_t = state_p.tile([PT, NS], f16, tag="T")
        U_t = state_p.tile([PT, NS], f16, tag="U")
        W_t = state_p.tile([PT, 2 * NANG], f32, tag="W")   # wrapped angles
        sg_t = state_p.tile([PT, 6 * 64], f32, tag="sg")   # +-1 sign rows
        s64_t = state_p.tile([PT, 64], f32, tag="s64")
        r64_t = state_p.tile([PT, 64], f32, tag="r64")

        # sign rows for <Z_w>, w = 0..5: blocks of 2^(5-w) alternate +1/-1
        for w in range(6):
            r = 1 << (5 - w)
            nc.vector.memset(sg_t[:, w * 64 : (w + 1) * 64], 1.0)
            neg = bass.AP(
                sg_t[:].tensor,
                sg_t[:].offset + w * 64 + r,
                [list(sg_t[:].ap)[0], [2 * r, 32 // r], [1, r]],
            )
            nc.vector.memset(neg, -1.0)

        # minimax-ish polynomial sin/cos on [-pi, pi] (t = y^2):
        # sin(y) = y * sum P[k] t^(4-k);  cos(y) = sum Q[k] t^(5-k)
        SIN_P = [2.2248706406891887e-06, -0.00019424154210166545,
                 0.008319842398281522, -0.16665145941120196,
                 0.9999972898367918]
        COS_Q = [-2.219394993734796e-07, 2.42531924958235e-05,
                 -0.001386274731586208, 0.04166103279007339,
                 -0.4999955816555398, 0.9999994436793969]
        mul_op, add_op = mybir.AluOpType.mult, mybir.AluOpType.add
        for t in range(NTILES):
            a_t = io_p.tile([PT, NANG], f32, tag="a")
            ang_t = io_p.tile([PT, NCOLS], f32, tag="ang")
            out6_t = io_p.tile([PT, NA], f32, tag="out6")
            nc.sync.dma_start(a_t[:], a_d[t * PT : (t + 1) * PT, :])

            # coefs on-chip: wrap to [-pi, pi], then Horner in y^2
            y, t2 = W_t[:, 0:NANG], W_t[:, NANG : 2 * NANG]
            aC = ang_t[:, 0:NANG]
            aS = ang_t[:, NANG : 2 * NANG]
            aNS = ang_t[:, 2 * NANG : 3 * NANG]
            nc.vector.add_range_wrap(y, a_t[:], 0.0, PI, 2.0 * PI)
            nc.vector.tensor_mul(t2, y, y)
            nc.vector.tensor_scalar(aS, t2, SIN_P[0], SIN_P[1], mul_op, add_op)
            for ck in SIN_P[2:]:
                nc.vector.tensor_mul(aS, aS, t2)
                nc.vector.tensor_scalar_add(aS, aS, ck)
            nc.vector.tensor_mul(aS, aS, y)
            nc.vector.tensor_scalar(aC, t2, COS_Q[0], COS_Q[1], mul_op, add_op)
            for ck in COS_Q[2:]:
                nc.vector.tensor_mul(aC, aC, t2)
                nc.vector.tensor_scalar_add(aC, aC, ck)
            nc.vector.tensor_scalar_mul(aNS, aS, -1.0)

            tiles = {"A": A_t[:], "B": B_t[:], "T": T_t[:], "U": U_t[:],
                     "S": s64_t[:]}
            nc.vector.memset(A_t[:], 0.0)
            nc.vector.memset(A_t[:, 0:1], 1.0)

            def scal(c):
                return ang_t[:, c : c + 1]

            def ap(reg):
                return _ap(bass, tiles[reg[0]], reg)

            for op in sched:
                kind = op[0]
                if kind == "rot2":
                    _, dst, s0, s1, c0, c1 = op
                    nc.vector._custom_dve(
                        cops["rot2"],
                        out=ap(dst), in0=ap(s0), in1=ap(s1),
                        s0=scal(c0), s1=scal(c1),
                    )
                elif kind == "cp":
                    _, dst, src = op
                    if os.environ.get("QK_CP", "rot2") == "rot2":
                        # copy as rot2 with immediate scalars (cheaper q than
                        # scalar.mul here); in1 = finite junk, scaled by 0
                        n = 1
                        for _, c in dst[2]:
                            n *= c
                        junk = (src[0], 0, ((1, n),))
                        nc.vector._custom_dve(
                            cops["rot2"],
                            out=ap(dst), in0=ap(src), in1=ap(junk),
                            s0=1.0, s1=0.0,
                        )
                    else:
                        nc.scalar.mul(ap(dst), ap(src), 1.0)
                elif kind == "sqsum":
                    _, dst, s0, s1 = op
                    nc.vector._custom_dve(
                        cops["sqsum"], out=ap(dst), in0=ap(s0), in1=ap(s1)
                    )
                elif kind == "red":
                    _, dst, src = op
                    nc.vector.tensor_reduce(
                        ap(dst), ap(src),
                        axis=mybir.AxisListType.X,
                        op=mybir.AluOpType.add,
                    )
                else:
                    raise ValueError(kind)
            # on-chip sign contraction: out6[:, w] = sum_b s64[b]*sg[w, b]
            # (tensor_tensor_reduce is broken in this runtime - wedges the
            # device - so mul + reduce instead)
            for w in range(NA):
                nc.vector.tensor_mul(
                    r64_t[:], s64_t[:], sg_t[:, w * 64 : (w + 1) * 64]
                )
                nc.vector.tensor_reduce(
                    out6_t[:, w : w + 1], r64_t[:],
                    axis=mybir.AxisListType.X, op=mybir.AluOpType.add,
                )
            nc.sync.dma_start(out_d[t * PT : (t + 1) * PT, :], out6_t[:])
    nc.compile()
    return nc


_NC_CACHE = None
_RUNNER = None


class _Result:
    exec_time_ns = None


class Runner:
    """Persistent jitted SPMD executor: same lowering as
    bass_utils.run_bass_kernel_spmd's axon path (bass2jax.run_bass_via_pjrt)
    but the jax.jit(shard_map(...)) closure is built once and cached, so
    warm calls skip retracing (~165ms/call)."""

    def __init__(self, nc, n_cores=NCORES):
        import jax
        from jax.sharding import Mesh, PartitionSpec
        from jax.experimental.shard_map import shard_map
        from concourse import bass2jax
        import concourse.mybir as mybir

        bass2jax.install_neuronx_cc_hook()
        self.nc = nc
        self.n_cores = n_cores
        part_name = nc.partition_id_tensor.name if nc.partition_id_tensor else None
        in_names, out_names, out_avals, self.zero_shapes = [], [], [], []
        for alloc in nc.m.functions[0].allocations:
            if not isinstance(alloc, mybir.MemoryLocationSet):
                continue
            name = alloc.memorylocations[0].name
            if alloc.kind == "ExternalInput":
                if name != part_name:
                    in_names.append(name)
            elif alloc.kind == "ExternalOutput":
                out_names.append(name)
                shape = tuple(alloc.tensor_shape)
                dtype = mybir.dt.np(alloc.dtype)
                out_avals.append(jax.core.ShapedArray(shape, dtype))
                self.zero_shapes.append((shape, dtype))
        self.in_names = list(in_names)
        self.out_names = list(out_names)
        self.out_avals = out_avals
        n_params = len(in_names)
        n_outs = len(out_names)
        all_in = list(in_names) + list(out_names)
        if part_name is not None:
            all_in.append(part_name)
        donate = tuple(range(n_params, n_params + n_outs))

        def _body(*args):
            operands = list(args)
            if nc.partition_id_tensor is not None:
                operands.append(bass2jax.partition_id_tensor())
            outs = bass2jax._bass_exec_p.bind(
                *operands,
                out_avals=tuple(out_avals),
                in_names=tuple(all_in),
                out_names=tuple(out_names),
                lowering_input_output_aliases=(),
                sim_require_finite=True,
                sim_require_nnan=True,
                nc=nc,
            )
            return tuple(outs)

        devices = jax.devices()[:n_cores]
        mesh = Mesh(np.asarray(devices), ("core",))
        in_specs = (PartitionSpec("core"),) * (n_params + n_outs)
        out_specs = (PartitionSpec("core"),) * n_outs
        self.fn = jax.jit(
            shard_map(_body, mesh=mesh, in_specs=in_specs,
                      out_specs=out_specs, check_rep=False),
            donate_argnums=donate,
            keep_unused=True,
        )

    def __call__(self, in_maps=None, concat_in=None):
        if concat_in is None:
            concat_in = [
                np.concatenate([m[name] for m in in_maps], axis=0)
                for name in self.in_names
            ]
        zeros = [
            np.zeros((self.n_cores * s[0], *s[1:]), d)
            for s, d in self.zero_shapes
        ]
        out_arrs = self.fn(*concat_in, *zeros)
        return [np.asarray(o) for o in out_arrs]  # global (n_cores*rows, ...)


def run_cores(a_full, trace=False, **kw):
    """a_full: (B, NANG) half-angles. Returns (B, NA) signed sums (no
    action scale/bias) + result handle."""
    global _NC_CACHE, _RUNNER
    if _NC_CACHE is None:
        _NC_CACHE = build_bass()
    nc = _NC_CACHE
    # the global (B, NANG) array IS the per-core concat (cores take
    # contiguous 256-row slices in order) - pass it straight through
    concat_in = [np.ascontiguousarray(a_full, dtype=np.float32)]
    last_err = None
    for attempt in range(3):
        try:
            if _RUNNER is None:
                _RUNNER = Runner(nc)
            results = _RUNNER(concat_in=concat_in)
            break
        except Exception as e:  # device occasionally needs a cooldown
            last_err = e
            import time as _time

            _time.sleep(45 * (attempt + 1))
    else:
        raise last_err
    s6 = results[0]  # "out" is the only output; already global (B, NA)
    return s6, _Result()


def kernel(x, input_scaling, weights, action_scale, action_bias):
    a = a_table(x, input_scaling, weights)
    s6, _ = run_cores(a)
    return s6 * np.asarray(action_scale, np.float32) + np.asarray(
        action_bias, np.float32
    )



# revision 2
# speedup vs baseline: 1.1713x; 1.1713x over previous
"""Trainium2 Bass kernel v2: 14-qubit data-reuploading quantum circuit actor.

Core idea vs v1: hand-authored 2x_1p custom-DVE *pair* ops on interleaved
complex fp16 — lo/hi lanes of the packed-fp16 datapath compute (re, im) of a
complex multiply, so a full merged per-wire gate U = RY(v)RZ(b)RY(a) is 4 fat
instructions at 2 elems/cycle:

    T   = U00 (x) X        (CMULIGN:  out = (C0+iC1) (x) in0)
    B.X = U01 (x) Y + T    (CMULACC:  out = (C0+iC1) (x) in0 + in1)
    T   = U10 (x) X
    B.Y = U11 (x) Y + T

U structure (alpha = RY input half-angle, beta = RZ half-angle incl weight-RZ,
vh = weight-RY half-angle): with p = alpha+vh, m = alpha-vh:
    A1 = cos(beta) cos(p); A2 = sin(beta) cos(m)
    B1 = cos(beta) sin(p); B2 = sin(beta) sin(m)
    U00 = A1 - i A2 ; U01 = -B1 + i B2 ; U10 = B1 + i B2 ; U11 = A1 + i A2

CNOT(t-1, t) of the ring folds into wire-t's write APs (region split on bit
t-1); CNOT(13, 0) folds into the next layer's wire-0 reads (and into the
measurement reads for the last layer). perf_max=1 is stamped on each pair-op
instruction post-Tile so the RTL engages the 2x_1p uop slot (validated on HW:
the 2x program's pair semantics only appear with perf_max=1).

Inputs shipped per call: raw x (2048 x 14 f32) + a 280-float aux table
(host-precomputed isc/2, weights/2 terms), ~120KB total vs 1.7MB for the v1
angle table. Angles + trig (range-wrap + Horner minimax) + the 6 coefficient
planes are computed on-chip per 128-row tile.
"""

import numpy as np

NQ = 14
NL = 5
OBS = 14
NA = 6
B = 2048
NCORES = 8
BPC = B // NCORES          # 256 rows per core
PT = 128                   # partitions per tile
NTILES = BPC // PT         # 2
NS = 1 << NQ               # 16384 amplitudes
F = 2 * NS                 # 32768 floats per row (interleaved complex)
NW = NL * NQ               # 70 (layer, wire) pairs
PI = float(np.pi)

# aux layout (floats, [1, 4*NW]): ISC1 | ISC2 | WT1 | VH
#   ISC1[l,w] = input_scaling[l,w]/2
#   ISC2[l,w] = input_scaling[l,w+14]/2
#   WT1[l,w]  = weights[l,w]/2
#   VH[l,w]   = weights[l,w+14]/2
NAUX = 4 * NW

SIN_P = [2.2248706406891887e-06, -0.00019424154210166545,
         0.008319842398281522, -0.16665145941120196,
         0.9999972898367918]
COS_Q = [-2.219394993734796e-07, 2.42531924958235e-05,
         -0.001386274731586208, 0.04166103279007339,
         -0.4999955816555398, 0.9999994436793969]


def make_aux(input_scaling, weights):
    isc = np.asarray(input_scaling, np.float64)
    wt = np.asarray(weights, np.float64)
    aux = np.concatenate([
        (isc[:, :NQ] / 2.0).ravel(),
        (isc[:, NQ:] / 2.0).ravel(),
        (wt[:, :NQ] / 2.0).ravel(),
        (wt[:, NQ:] / 2.0).ravel(),
    ]).astype(np.float32)
    return aux.reshape(1, NAUX)


def coef_planes(x, aux):
    """Host/numpy mirror of the on-chip coef computation (float64 path).
    x: (n, 14) -> dict of (n, 70) planes A1, A2, B1, B2."""
    x = np.asarray(x, np.float64)
    a = np.asarray(aux, np.float64).ravel()
    isc1 = a[0:NW].reshape(NL, NQ)
    isc2 = a[NW:2 * NW].reshape(NL, NQ)
    wt1 = a[2 * NW:3 * NW].reshape(NL, NQ)
    vh = a[3 * NW:4 * NW].reshape(NL, NQ)
    xb = x[:, None, :]
    alpha = isc1[None] * xb
    beta = isc2[None] * xb + wt1[None]
    p = alpha + vh[None]
    m = alpha - vh[None]
    cb, sb = np.cos(beta), np.sin(beta)
    return {
        "A1": (cb * np.cos(p)).reshape(-1, NW).astype(np.float32),
        "A2": (sb * np.cos(m)).reshape(-1, NW).astype(np.float32),
        "B1": (cb * np.sin(p)).reshape(-1, NW).astype(np.float32),
        "B2": (sb * np.sin(m)).reshape(-1, NW).astype(np.float32),
    }


# ---------------------------------------------------------------- schedule
# region = (buf, float_offset, dims); dims = ((step, count), ...) innermost
# last, float-index space. buf: "A"/"B" state, "T" scratch (16384 floats),
# "S" s64 sums. Every pair-op region: innermost step 1, even count, even
# offset (2x_1p eligibility).
#
# ops:
#  ("cmulign", dst, src, c0, c1): dst = (c0+ic1) (x) src      [in1 ignored]
#  ("cmulacc", dst, src, acc, c0, c1): dst = (c0+ic1)(x)src + acc (acc rank-1)
#  ("sqsum", dst, s0, s1): dst = s0^2 + s1^2
#  ("red", dst, src): 64-block reduce
# scalar ref = (plane, col), plane in A1,A2,B1,B2,NA2,NB1.


def _norm(dims):
    d = [(s, c) for s, c in dims if c != 1]
    out = []
    for s, c in d:
        if out and out[-1][0] == s * c:
            out[-1] = (s, c * out[-1][1])
        else:
            out.append((s, c))
    if not out:
        out = [(1, 1)]
    assert len(out) <= 2, out
    return tuple(out)


def _reg(buf, off, *dims):
    return (buf, off, _norm(dims))


def _nelem(reg):
    n = 1
    for _, c in reg[2]:
        n *= c
    return n


class Sched:
    def __init__(self):
        self.ops = []
        self.cur = "A"

    def swap(self):
        self.cur = "B" if self.cur == "A" else "A"

    def gate(self, l, t):
        """Merged U(l, t) with ring-fold on writes (t>=1) and prev-layer
        C(13,0) fold on reads (t==0, l>=1)."""
        a, b = self.cur, "B" if self.cur == "A" else "A"
        col = l * NQ + t
        A1, A2 = ("A1", col), ("A2", col)
        B1, B2 = ("B1", col), ("B2", col)
        NA2, NB1 = ("NA2", col), ("NB1", col)

        if t == 0:
            if l == 0:
                X = [_reg(a, 0, (1, NS))]
                Y = [_reg(a, NS, (1, NS))]
                DX = [_reg(b, 0, (1, NS))]
                DY = [_reg(b, NS, (1, NS))]
            else:
                d = ((4, NS // 4), (1, 2))
                X = [_reg(a, 0, *d), _reg(a, NS + 2, *d)]
                Y = [_reg(a, NS, *d), _reg(a, 2, *d)]
                DX = [_reg(b, 0, *d), _reg(b, 2, *d)]
                DY = [_reg(b, NS, *d), _reg(b, NS + 2, *d)]
        else:
            Ft = 1 << (14 - t)
            nb = 1 << (t - 1)
            d = ((4 * Ft, nb), (1, Ft))
            X = [_reg(a, 0, *d), _reg(a, 2 * Ft, *d)]
            Y = [_reg(a, Ft, *d), _reg(a, 3 * Ft, *d)]
            # ring C(t-1, t): odd-b (bit t-1 = 1) writes land bit-t-flipped
            DX = [_reg(b, 0, *d), _reg(b, 3 * Ft, *d)]    # out0 -> X | Yo
            DY = [_reg(b, Ft, *d), _reg(b, 2 * Ft, *d)]   # out1 -> Y | Xo

        nparts = len(X)
        half = NS if nparts == 1 else NS // 2
        for i in range(nparts):
            Ti = _reg("T", i * half, (1, half))
            self.ops.append(("cmulign", Ti, X[i], A1, NA2))        # U00 (x) X
            self.ops.append(("cmulacc", DX[i], Y[i], Ti, NB1, B2))  # +U01 (x) Y
        for i in range(nparts):
            Ti = _reg("T", i * half, (1, half))
            self.ops.append(("cmulign", Ti, X[i], B1, B2))          # U10 (x) X
            self.ops.append(("cmulacc", DY[i], Y[i], Ti, A1, A2))   # +U11 (x) Y
        self.swap()

    def layer0_build(self):
        """Layer-1 on |0..0>: product state via doubling, appending qubit w
        as the new innermost index; ring C(w-1, w) folds into the append APs
        (odd source index j <-> bit w-1 = 1 -> flip new bit w).
        s_0..s_11 ping-pong in T halves, s_12 -> B[0:16384], s_13 -> A."""
        assert self.cur == "A"
        ops = self.ops
        # seed: T[0:2] = (1, 0) — emitted by the bass builder (memset), and
        # by the numpy executor, via the special op below.
        ops.append(("seed",))

        def v0(w):
            return ("A1", w), ("NA2", w)   # U00 column entry

        def v1(w):
            return ("B1", w), ("B2", w)    # U10

        def place(k):
            # buffer holding s_k (size 2**(k+2) floats)
            if k <= 11:
                return ("T", 8192 * (k % 2))
            if k == 12:
                return ("B", 0)
            return ("A", 0)

        # qubit 0: s_0 from seed (no fold)
        c0, s0 = v0(0)
        c1, s1 = v1(0)
        seed = _reg("T", 0, (1, 2))
        dstb, dsto = place(0)
        # b=1 first (disjoint), then b=0 in-place over the seed
        ops.append(("cmulign", _reg(dstb, dsto + 2, (4, 1), (1, 2)), seed, c1, s1))
        ops.append(("cmulign", _reg(dstb, dsto + 0, (4, 1), (1, 2)), seed, c0, s0))
        for w in range(1, NQ):
            sb, so = place(w - 1)
            db, do = place(w)
            nE = 1 << (w - 1)  # even-j count == odd-j count
            srcE = _reg(sb, so + 0, (4, nE), (1, 2))
            srcO = _reg(sb, so + 2, (4, nE), (1, 2))
            c0, s0 = v0(w)
            c1, s1 = v1(w)
            # b=0 (U00 factor): even j -> 2j ; odd j -> 2j+1 (bit-w flip)
            ops.append(("cmulign", _reg(db, do + 0, (8, nE), (1, 2)), srcE, c0, s0))
            ops.append(("cmulign", _reg(db, do + 6, (8, nE), (1, 2)), srcO, c0, s0))
            # b=1 (U10 factor): even j -> 2j+1 ; odd j -> 2j
            ops.append(("cmulign", _reg(db, do + 2, (8, nE), (1, 2)), srcE, c1, s1))
            ops.append(("cmulign", _reg(db, do + 4, (8, nE), (1, 2)), srcO, c1, s1))
        # s_13 landed in A; cur stays "A"

    def measurement(self):
        a = self.cur
        self.ops.append(("sqsum", _reg("T", 0, (2, 8192)),
                         _reg(a, 0, (4, 8192)), _reg(a, 1, (4, 8192))))
        self.ops.append(("sqsum", _reg("T", 1, (2, 4096)),
                         _reg(a, NS + 2, (4, 4096)), _reg(a, NS + 3, (4, 4096))))
        self.ops.append(("sqsum", _reg("T", 8193, (2, 4096)),
                         _reg(a, 2, (4, 4096)), _reg(a, 3, (4, 4096))))
        self.ops.append(("red", ("S", 0, ((1, 64),)),
                         ("T", 0, ((256, 64), (1, 256)))))


def build_schedule():
    S = Sched()
    S.layer0_build()
    for l in range(1, NL):
        for t in range(NQ):
            S.gate(l, t)
    S.measurement()
    return S.ops


# ------------------------------------------------------------ numpy executor


def _indices(reg):
    _, off, dims = reg
    idx = np.array([0], np.int64)
    for st, ct in dims:
        idx = (idx[:, None] + (np.arange(ct, dtype=np.int64) * st)[None, :]).ravel()
    return off + idx


def simulate_numpy(x, aux, fp16=True):
    """x: (n, 14) -> (n, 64) block sums, mirroring the on-device schedule."""
    pl = coef_planes(x, aux)
    pl = dict(pl)
    pl["NA2"] = -pl["A2"]
    pl["NB1"] = -pl["B1"]
    n = x.shape[0]
    sdt = np.float16 if fp16 else np.float32
    bufs = {
        "A": np.zeros((n, F), sdt),
        "B": np.zeros((n, F), sdt),
        "T": np.zeros((n, NS), sdt),
        "S": np.zeros((n, 64), np.float32),
    }
    bufs["A"][:, 0] = 1.0

    def cmul(src_v, c, s):
        lo, hi = src_v[:, 0::2], src_v[:, 1::2]
        out = np.empty_like(src_v)
        out[:, 0::2] = c * lo - s * hi
        out[:, 1::2] = s * lo + c * hi
        return out

    for op in build_schedule():
        kind = op[0]
        if kind == "seed":
            bufs["T"][:, 0] = 1.0
            bufs["T"][:, 1] = 0.0
        elif kind in ("cmulign", "cmulacc"):
            if kind == "cmulign":
                _, dst, src, c0, c1 = op
                acc_v = 0.0
            else:
                _, dst, src, acc, c0, c1 = op
                acc_v = bufs[acc[0]][:, _indices(acc)].astype(np.float32)
            c = pl[c0[0]][:n, c0[1]:c0[1] + 1].astype(np.float32)
            s = pl[c1[0]][:n, c1[1]:c1[1] + 1].astype(np.float32)
            src_v = bufs[src[0]][:, _indices(src)].astype(np.float32)
            v = cmul(src_v, c, s) + acc_v
            bufs[dst[0]][:, _indices(dst)] = v.astype(sdt)
        elif kind == "sqsum":
            _, dst, s0, s1 = op
            v = (bufs[s0[0]][:, _indices(s0)].astype(np.float32) ** 2
                 + bufs[s1[0]][:, _indices(s1)].astype(np.float32) ** 2)
            bufs[dst[0]][:, _indices(dst)] = v.astype(sdt)
        elif kind == "red":
            _, dst, src = op
            v = bufs[src[0]][:, _indices(src)].astype(np.float32)
            bufs["S"][:, _indices(dst)] = v.reshape(n, 64, 256).sum(axis=2)
        else:
            raise ValueError(kind)
    return bufs["S"].copy()


def postprocess(s64, action_scale, action_bias):
    blk = np.arange(64)
    out = np.zeros((s64.shape[0], NA), np.float32)
    for w in range(NA):
        sign = 1.0 - 2.0 * ((blk >> (5 - w)) & 1)
        out[:, w] = s64 @ sign.astype(np.float32)
    return out * np.asarray(action_scale, np.float32) + np.asarray(
        action_bias, np.float32)


# ------------------------------------------------------------------ DVE ops

_CUSTOM = {}


def _build_pair_uop(with_acc):
    from concourse.dve_uop import (
        InpSel, OutSel, AluInp as D, DelayInp, OutPath, Trigger, UopConfig,
        UopDpConfig, AluOp, ENABLE)

    def dp(op, a, b, capture=None, passes=()):
        d = UopDpConfig().enable_alu(op, a, b)
        if capture is not None:
            d.enable_delay_from_src(DelayInp.PREV_ALU_OUT, capture)
        if passes:
            d.pass_through_delay(*passes)
        return d

    u = UopConfig()
    u.enable_input(InpSel.SRC_0, 1)      # d0 = X_lo
    u.enable_input(InpSel.CONST_0, 2)    # d1 = C0
    u.enable_input(InpSel.SRC_0_HI, 3)   # d2 = X_hi
    u.enable_input(InpSel.CONST_1, 4)    # d3 = C1
    u.enable_input(InpSel.SRC_1, 5)      # d4 = T_lo
    u.enable_input(InpSel.SRC_1_HI, 6)   # d5 = T_hi
    if with_acc:
        u.datapath_config[0] = dp(AluOp.MULTIPLY, D.PREV_DELAY_0, D.PREV_DELAY_1,
                                  passes=(0, 1, 2, 3, 4, 5))
        u.datapath_config[1] = dp(AluOp.ADD, D.PREV_ALU_OUT, D.PREV_DELAY_4,
                                  passes=(0, 1, 2, 3, 5))
        u.datapath_config[2] = dp(AluOp.MULTIPLY, D.PREV_DELAY_2, D.PREV_DELAY_3,
                                  capture=4, passes=(0, 1, 2, 3, 5))
        u.datapath_config[3] = dp(AluOp.SUBTRACT, D.PREV_DELAY_4, D.PREV_ALU_OUT,
                                  passes=(0, 1, 2, 3, 5))
        u.datapath_config[4] = dp(AluOp.MULTIPLY, D.PREV_DELAY_0, D.PREV_DELAY_3,
                                  capture=4, passes=(1, 2, 5))
        u.datapath_config[5] = dp(AluOp.MULTIPLY, D.PREV_DELAY_2, D.PREV_DELAY_1,
                                  capture=0, passes=(4, 5))
        u.datapath_config[6] = dp(AluOp.ADD, D.PREV_DELAY_0, D.PREV_ALU_OUT,
                                  passes=(4, 5))
        u.datapath_config[7] = dp(AluOp.ADD, D.PREV_ALU_OUT, D.PREV_DELAY_5,
                                  passes=(4,))
    else:
        u.datapath_config[0] = dp(AluOp.MULTIPLY, D.PREV_DELAY_0, D.PREV_DELAY_1,
                                  passes=(0, 1, 2, 3))
        u.datapath_config[1] = dp(AluOp.MULTIPLY, D.PREV_DELAY_2, D.PREV_DELAY_3,
                                  capture=4, passes=(0, 1, 2, 3))
        u.datapath_config[2] = dp(AluOp.SUBTRACT, D.PREV_DELAY_4, D.PREV_ALU_OUT,
                                  passes=(0, 1, 2, 3))
        u.datapath_config[3] = dp(AluOp.MULTIPLY, D.PREV_DELAY_0, D.PREV_DELAY_3,
                                  capture=4, passes=(1, 2))
        u.datapath_config[4] = dp(AluOp.MULTIPLY, D.PREV_DELAY_2, D.PREV_DELAY_1,
                                  capture=5, passes=(4,))
        u.datapath_config[5] = dp(AluOp.ADD, D.PREV_DELAY_5, D.PREV_ALU_OUT,
                                  passes=(4,))
        u.datapath_config[6] = dp(AluOp.BYPASS, D.PREV_ALU_OUT, D.PREV_ALU_OUT,
                                  passes=(4,))
        u.datapath_config[7] = dp(AluOp.BYPASS, D.PREV_ALU_OUT, D.PREV_ALU_OUT,
                                  passes=(4,))
    u.enable_output(OutSel.DELAY_4, OutPath.WR0_LO)
    u.enable_output(OutSel.ALU_OUT, OutPath.WR0_HI)
    u.require_inp0 = ENABLE
    u.require_inp1 = ENABLE
    u.trigger = (Trigger.SRC_TENSOR_DONE, Trigger.NONE, Trigger.NONE)
    u.next_uop = (0, 0, 0)
    return u


def _sc_np(s, p):
    s = np.asarray(s, np.float32)
    return s.reshape(p, -1) if s.size > 1 else s.reshape(-1)


def _cmulacc_ref(in0, in1, s0, s1, imm2):
    p = in0.shape[0]
    x = np.asarray(in0, np.float32).reshape(p, -1)
    t = np.asarray(in1, np.float32).reshape(p, -1)
    c, s = _sc_np(s0, p), _sc_np(s1, p)
    out = np.empty_like(x)
    out[:, 0::2] = c * x[:, 0::2] - s * x[:, 1::2] + t[:, 0::2]
    out[:, 1::2] = s * x[:, 0::2] + c * x[:, 1::2] + t[:, 1::2]
    return out.reshape(in0.shape)


def _cmulign_ref(in0, in1, s0, s1, imm2):
    p = in0.shape[0]
    x = np.asarray(in0, np.float32).reshape(p, -1)
    c, s = _sc_np(s0, p), _sc_np(s1, p)
    out = np.empty_like(x)
    out[:, 0::2] = c * x[:, 0::2] - s * x[:, 1::2]
    out[:, 1::2] = s * x[:, 0::2] + c * x[:, 1::2]
    return out.reshape(in0.shape)


def _sqsum_ref(in0, in1, s0, s1, imm2):
    p = in0.shape[0]
    a = np.asarray(in0, np.float32).reshape(p, -1)
    b = np.asarray(in1, np.float32).reshape(p, -1)
    return (a * a + b * b).reshape(in0.shape)


def _get_custom_ops():
    if _CUSTOM:
        return _CUSTOM
    from concourse import dve_ops
    from concourse.dve_ops import DveOp, OPS
    from concourse.dve_spec import Spec, Src0, Src1, C0, C1, sq, lower
    from concourse.dve_uop import DveOpSpec

    _SPEC_CACHE = {}

    def register(name, body, ref, uop2x):
        for op in OPS:
            if op.name == name:
                return op
        row = dve_ops._CUSTOM_DVE_ROW_BASE + len(OPS)
        spec = Spec(body=body, reference=ref)

        if uop2x is not None:
            class DveOpPair(DveOp):
                def compile(self, ver):
                    key = (self.name, ver)
                    if key in _SPEC_CACHE:
                        return _SPEC_CACHE[key]
                    s = DveOpSpec(
                        name=self.name,
                        opcode=dve_ops.get_dve_sub_opcode(self.name),
                        uops=lower(self.spec, ver=ver),
                        uops_2x=[uop2x],
                        perf_max=1,
                        rd1_en=True,
                    )
                    got = s.sha(ver)
                    if self.uops_sha.get(ver) != got:
                        raise ValueError(f"{self.name}: sha drift {got}")
                    _SPEC_CACHE[key] = s
                    return s
            cls = DveOpPair
        else:
            cls = DveOp
        shas = {}
        for ver in ("v3", "v4"):
            kw = dict(uops_2x=[uop2x], perf_max=1) if uop2x is not None else {}
            s = DveOpSpec(name=name, opcode=row, uops=lower(spec, ver=ver),
                          rd1_en=True, **kw)
            shas[ver] = s.sha(ver)
        op = cls(name, spec, subdim=False, uops_sha=shas)
        OPS.append(op)
        dve_ops._SUB_OPCODE_FOR_NAME[name] = row
        dve_ops.CUSTOM_DVE_SPECS[name] = spec
        return op

    # 1x placeholder bodies are flat (wrong for pair semantics) — correctness
    # depends on the 2x slot engaging; emitter asserts AP eligibility.
    _CUSTOM["cmulacc"] = register(
        "CMULACC_K", Src0 * C0 + Src1 * C1, _cmulacc_ref, _build_pair_uop(True))
    _CUSTOM["cmulign"] = register(
        "CMULIGN_K", Src0 * C0 + Src1 * C1, _cmulign_ref, _build_pair_uop(False))
    _CUSTOM["sqsum"] = register(
        "SQSUM_K", sq(Src0) + sq(Src1), _sqsum_ref, None)
    return _CUSTOM


# ------------------------------------------------------------------ bass side


def _ap(bass_mod, tile_ap, reg):
    t = tile_ap.tensor
    part = list(tile_ap.ap)[0]
    dims = [[part[0], part[1]]] + [[s, c] for s, c in reg[2]]
    return bass_mod.AP(t, tile_ap.offset + reg[1], dims)


def _check_pair_eligible(reg):
    _, off, dims = reg
    assert off % 2 == 0, reg
    st, ct = dims[-1]
    assert st == 1 and ct >= 2 and ct % 2 == 0, reg
    if len(dims) == 2:
        assert dims[0][0] % 2 == 0, reg


def build_bass():
    import concourse.bass as bass
    import concourse.mybir as mybir
    import concourse.tile as tile
    from concourse import bacc
    from contextlib import ExitStack

    f32 = mybir.dt.float32
    f16 = mybir.dt.float16
    cops = _get_custom_ops()
    sched = build_schedule()
    mul_op, add_op = mybir.AluOpType.mult, mybir.AluOpType.add

    nc = bacc.Bacc("TRN2", target_bir_lowering=False, debug=False)
    x_d = nc.dram_tensor("x", [BPC, OBS], f32, kind="ExternalInput").ap()
    aux_d = nc.dram_tensor("aux", [1, NAUX], f32, kind="ExternalInput").ap()
    out_d = nc.dram_tensor("out", [BPC, NA], f32, kind="ExternalOutput").ap()

    pm_names = []

    def emit_pair(kind, dst_ap, src_ap, in1_ap, s0, s1):
        inst = nc.vector._custom_dve(
            cops[kind], out=dst_ap, in0=src_ap, in1=in1_ap, s0=s0, s1=s1)
        raw = inst.ins if hasattr(inst, "ins") else inst
        pm_names.append(raw.name)
        return inst

    with tile.TileContext(nc) as tc, ExitStack() as ctx:
        state_p = ctx.enter_context(tc.tile_pool(name="state", bufs=1))
        io_p = ctx.enter_context(tc.tile_pool(name="io", bufs=2))

        A_t = state_p.tile([PT, F], f16, tag="A")
        B_t = state_p.tile([PT, F], f16, tag="B")
        T_t = state_p.tile([PT, NS], f16, tag="T")
        aux_t = state_p.tile([PT, NAUX], f32, tag="aux")
        ANG_t = state_p.tile([PT, 6 * NW], f32, tag="ang")  # y(210) | t2(210)
        CS_t = state_p.tile([PT, 6 * NW], f32, tag="cs")   # cos(210) | sin(210)
        PL_t = state_p.tile([PT, 6 * NW], f32, tag="pl")   # A1 A2 B1 B2 NA2 NB1
        W_t = state_p.tile([PT, 3 * NW], f32, tag="w")     # raw angles p|m|beta
        sg_t = state_p.tile([PT, 6 * 64], f32, tag="sg")
        s64_t = state_p.tile([PT, 64], f32, tag="s64")
        r64_t = state_p.tile([PT, 64], f32, tag="r64")

        # aux broadcast to all partitions: 1 DMA + 7 doubling DMAs
        nc.sync.dma_start(aux_t[0:1, :], aux_d)
        k = 1
        while k < PT:
            nc.sync.dma_start(aux_t[k:2 * k, :], aux_t[0:k, :])
            k *= 2

        # sign rows for <Z_w>
        for w in range(6):
            r = 1 << (5 - w)
            nc.vector.memset(sg_t[:, w * 64:(w + 1) * 64], 1.0)
            neg = bass.AP(
                sg_t[:].tensor, sg_t[:].offset + w * 64 + r,
                [list(sg_t[:].ap)[0], [2 * r, 32 // r], [1, r]])
            nc.vector.memset(neg, -1.0)

        PLANE = {"A1": 0, "A2": 1, "B1": 2, "B2": 3, "NA2": 4, "NB1": 5}

        for tno in range(NTILES):
            x_t = io_p.tile([PT, OBS], f32, tag="x")
            out6_t = io_p.tile([PT, NA], f32, tag="out6")
            nc.sync.dma_start(x_t[:], x_d[tno * PT:(tno + 1) * PT, :])

            # --- coefficient planes ---------------------------------------
            # alpha[l,w] = ISC1*x ; beta = ISC2*x + WT1 ; p/m = alpha +- VH
            X5 = W_t[:, 0:NW]      # temp: x tiled 5x
            for l in range(NL):
                nc.vector.tensor_copy(X5[:, l * NQ:(l + 1) * NQ], x_t[:])
            alpha = ANG_t[:, 0:NW]  # temp
            nc.vector.tensor_mul(alpha, X5, aux_t[:, 0:NW])
            beta = W_t[:, 2 * NW:3 * NW]
            nc.vector.tensor_mul(beta, X5, aux_t[:, NW:2 * NW])
            nc.vector.tensor_add(beta, beta, aux_t[:, 2 * NW:3 * NW])
            p_ = W_t[:, 0:NW]      # overwrites X5 (alpha already extracted)
            m_ = W_t[:, NW:2 * NW]
            nc.vector.tensor_add(p_, alpha, aux_t[:, 3 * NW:4 * NW])
            nc.vector.tensor_sub(m_, alpha, aux_t[:, 3 * NW:4 * NW])

            # trig over [p | m | beta] (210 cols): wrap + Horner
            y = ANG_t[:, 0:3 * NW]
            t2 = ANG_t[:, 3 * NW:6 * NW]
            aC = CS_t[:, 0:3 * NW]
            aS = CS_t[:, 3 * NW:6 * NW]
            nc.vector.add_range_wrap(y, W_t[:, 0:3 * NW], 0.0, PI, 2.0 * PI)
            nc.vector.tensor_mul(t2, y, y)
            nc.vector.tensor_scalar(aS, t2, SIN_P[0], SIN_P[1], mul_op, add_op)
            for ck in SIN_P[2:]:
                nc.vector.tensor_mul(aS, aS, t2)
                nc.vector.tensor_scalar_add(aS, aS, ck)
            nc.vector.tensor_mul(aS, aS, y)
            nc.vector.tensor_scalar(aC, t2, COS_Q[0], COS_Q[1], mul_op, add_op)
            for ck in COS_Q[2:]:
                nc.vector.tensor_mul(aC, aC, t2)
                nc.vector.tensor_scalar_add(aC, aC, ck)

            cosp, cosm, cosb = (aC[:, 0:NW], aC[:, NW:2 * NW], aC[:, 2 * NW:3 * NW])
            sinp, sinm, sinb = (aS[:, 0:NW], aS[:, NW:2 * NW], aS[:, 2 * NW:3 * NW])
            A1 = PL_t[:, 0:NW]
            A2 = PL_t[:, NW:2 * NW]
            B1 = PL_t[:, 2 * NW:3 * NW]
            B2 = PL_t[:, 3 * NW:4 * NW]
            NA2v = PL_t[:, 4 * NW:5 * NW]
            NB1v = PL_t[:, 5 * NW:6 * NW]
            nc.vector.tensor_mul(A1, cosb, cosp)
            nc.vector.tensor_mul(A2, sinb, cosm)
            nc.vector.tensor_mul(B1, cosb, sinp)
            nc.vector.tensor_mul(B2, sinb, sinm)
            nc.vector.tensor_scalar_mul(NA2v, A2, -1.0)
            nc.vector.tensor_scalar_mul(NB1v, B1, -1.0)

            # --- state init + gates ---------------------------------------
            nc.vector.memset(A_t[:], 0.0)
            nc.vector.memset(A_t[:, 0:1], 1.0)

            tiles = {"A": A_t[:], "B": B_t[:], "T": T_t[:], "S": s64_t[:]}

            def ap(reg):
                return _ap(bass, tiles[reg[0]], reg)

            def scal(ref):
                pli, col = PLANE[ref[0]], ref[1]
                return PL_t[:, pli * NW + col:pli * NW + col + 1]

            for op in sched:
                kind = op[0]
                if kind == "seed":
                    nc.vector.memset(T_t[:, 0:1], 1.0)
                    nc.vector.memset(T_t[:, 1:2], 0.0)
                elif kind == "cmulign":
                    _, dst, src, c0, c1 = op
                    _check_pair_eligible(dst)
                    _check_pair_eligible(src)
                    n = _nelem(src)
                    dummy = ("A", 0, ((1, n),))
                    emit_pair("cmulign", ap(dst), ap(src), ap(dummy),
                              scal(c0), scal(c1))
                elif kind == "cmulacc":
                    _, dst, src, acc, c0, c1 = op
                    _check_pair_eligible(dst)
                    _check_pair_eligible(src)
                    _check_pair_eligible(acc)
                    assert len(acc[2]) == 1
                    emit_pair("cmulacc", ap(dst), ap(src), ap(acc),
                              scal(c0), scal(c1))
                elif kind == "sqsum":
                    _, dst, s0, s1 = op
                    nc.vector._custom_dve(
                        cops["sqsum"], out=ap(dst), in0=ap(s0), in1=ap(s1))
                elif kind == "red":
                    _, dst, src = op
                    nc.vector.tensor_reduce(
                        ap(dst), ap(src), axis=mybir.AxisListType.X,
                        op=mybir.AluOpType.add)
                else:
                    raise ValueError(kind)

            for w in range(NA):
                nc.vector.tensor_mul(
                    r64_t[:], s64_t[:], sg_t[:, w * 64:(w + 1) * 64])
                nc.vector.tensor_reduce(
                    out6_t[:, w:w + 1], r64_t[:],
                    axis=mybir.AxisListType.X, op=mybir.AluOpType.add)
            nc.sync.dma_start(out_d[tno * PT:(tno + 1) * PT, :], out6_t[:])

    # stamp perf_max=1 post-Tile (scheduling rebuilds instructions)
    names = set(pm_names)
    n_pm = 0
    for fn in nc.m.functions:
        for blk in fn.blocks:
            for inst in blk.instructions:
                if type(inst).__name__ == "InstCustomDveAnt" and inst.name in names:
                    inst.perf_max = 1
                    n_pm += 1
    assert n_pm == len(names), (n_pm, len(names))
    nc.compile()
    return nc


# ------------------------------------------------------------------- runner

_NC_CACHE = None
_RUNNER = None


class _Result:
    exec_time_ns = None


class Runner:
    """Persistent jitted SPMD executor (cached shard_map closure)."""

    def __init__(self, nc, n_cores=NCORES):
        import jax
        from jax.sharding import Mesh, PartitionSpec
        from jax.experimental.shard_map import shard_map
        from concourse import bass2jax
        import concourse.mybir as mybir

        bass2jax.install_neuronx_cc_hook()
        self.nc = nc
        self.n_cores = n_cores
        part_name = nc.partition_id_tensor.name if nc.partition_id_tensor else None
        in_names, out_names, out_avals, self.zero_shapes = [], [], [], []
        for alloc in nc.m.functions[0].allocations:
            if not isinstance(alloc, mybir.MemoryLocationSet):
                continue
            name = alloc.memorylocations[0].name
            if alloc.kind == "ExternalInput":
                if name != part_name:
                    in_names.append(name)
            elif alloc.kind == "ExternalOutput":
                out_names.append(name)
                shape = tuple(alloc.tensor_shape)
                dtype = mybir.dt.np(alloc.dtype)
                out_avals.append(jax.core.ShapedArray(shape, dtype))
                self.zero_shapes.append((shape, dtype))
        self.in_names = list(in_names)
        self.out_names = list(out_names)
        n_params = len(in_names)
        n_outs = len(out_names)
        all_in = list(in_names) + list(out_names)
        if part_name is not None:
            all_in.append(part_name)
        donate = tuple(range(n_params, n_params + n_outs))

        def _body(*args):
            operands = list(args)
            if nc.partition_id_tensor is not None:
                operands.append(bass2jax.partition_id_tensor())
            outs = bass2jax._bass_exec_p.bind(
                *operands,
                out_avals=tuple(out_avals),
                in_names=tuple(all_in),
                out_names=tuple(out_names),
                lowering_input_output_aliases=(),
                sim_require_finite=True,
                sim_require_nnan=True,
                nc=nc,
            )
            return tuple(outs)

        devices = jax.devices()[:n_cores]
        mesh = Mesh(np.asarray(devices), ("core",))
        in_specs = (PartitionSpec("core"),) * (n_params + n_outs)
        out_specs = (PartitionSpec("core"),) * n_outs
        self.fn = jax.jit(
            shard_map(_body, mesh=mesh, in_specs=in_specs,
                      out_specs=out_specs, check_rep=False),
            donate_argnums=donate,
            keep_unused=True,
        )

    def __call__(self, concat_in):
        zeros = [
            np.zeros((self.n_cores * s[0], *s[1:]), d)
            for s, d in self.zero_shapes
        ]
        out_arrs = self.fn(*concat_in, *zeros)
        return [np.asarray(o) for o in out_arrs]


def run_cores(x_full, aux):
    global _NC_CACHE, _RUNNER
    if _NC_CACHE is None:
        _NC_CACHE = build_bass()
    nc = _NC_CACHE
    xs = np.ascontiguousarray(x_full, np.float32)
    aux_rep = np.ascontiguousarray(
        np.broadcast_to(aux.reshape(1, NAUX), (NCORES, NAUX)), np.float32)
    last_err = None
    for attempt in range(3):
        try:
            if _RUNNER is None:
                _RUNNER = Runner(nc)
            results = _RUNNER([xs, aux_rep])
            break
        except Exception as e:
            last_err = e
            import time as _time
            _time.sleep(45 * (attempt + 1))
    else:
        raise last_err
    return results[0], _Result()


def kernel(x, input_scaling, weights, action_scale, action_bias):
    aux = make_aux(input_scaling, weights)
    s6, _ = run_cores(np.asarray(x, np.float32), aux)
    return s6 * np.asarray(action_scale, np.float32) + np.asarray(
        action_bias, np.float32)
